# revision 21
# baseline (speedup 1.0000x reference)
"""Trainium2 Bass kernel for nn_Blast: out = x @ (W0 + 1 bias^T) + bias
where W0 block (i_in, i_out) = Vt[i] @ diag(S[o,i]) @ U[o].

v3: full bf16 dataflow (tolerance is 2e-2; bf16 end-to-end lands ~4e-3).

Per core (256 tokens):
  midT[(o,r), tok] = sum_in A[in, (o,r)] * xT[in, tok]     (A = Vt*S, built on device)
  out[tok, oq]     = sum_r midT[(o,r), tok] * U'[o, r, q]

Mid rows pack o-blocks as bank = o%3, slot = o//3: 16 rank rows per slot plus
a shared rowsum row (112; A ones-column -> bias*(rowsum+1) supplies both bias
terms) and a comp row (113; cancels the +1.0 bank-open pollution). Mid width
is 114 but A banks are padded to 128 columns so the stationary operand is a
full 128 - that enables FWL fast weight loads (bf16, P=128), hiding LDWEIGHTS
entirely. Phase A = 32 K-chunks x 3 matmuls (N=256, M=128).

The S-row stagings (PE broadcast of S to 128 partitions, read by the DVE/GPS
A-builds straight from PSUM) are interleaved INTO the phase-A matmul stream
(stage for i-block i rides with chunk 2i-8), keeping them off the critical
path and doubling as HAM keep-warm activity. A few dummy warm matmuls run
before phase A; chunk matmuls for the first chunks get fillers to hold the
2.4 GHz clock through the DMA ramp.

Phase B reads each mid bank wholesale (K=114) against the sparse stacked-U
matrix usb[16s+r, 256o+q] = U[o,r,q] iff s==o//3: one matmul per o-block
(N=256), paired two per PSUM tile so PSUM->SBUF copies run at [128,512].
The mod-3 bank map makes each usb 16-row slot group cover contiguous
o-blocks, so U loads with 6 plain DMAs - no on-device scatter.

DMA (shared 16-SDMA pool, ~358 GB/s/core): x 2MB on sync ring; vt/s then
uc/bc on scalar ring; aship on gpsimd; out 2MB in 3 pieces per token-half
split across sync+gpsimd so the tail is one 256KB transfer.
"""

import numpy as np

IN_DIM = 4096
OUT_DIM = 4096
BLOCK = 256
RANK = 16
B_IN = 16
B_OUT = 16
N_CORES = 8
TOK = 2048
TPC = TOK // N_CORES          # 256 tokens per core
NCHUNK = IN_DIM // 128        # 32 K-chunks
W = 114                       # mid-bank width: 7*16 rank + rowsum + comp
NB = 3                        # mid banks, o-block -> bank o%3 slot o//3
SW = NB * W                   # 342 A-columns per chunk
SSW = SW                      # staged-S columns per i-block
KSHIP = 4                     # prebuilt A chunks shipped from host
NWARM = 12                    # PE warm matmuls before phase A
GPS_BUILD = {19, 21, 23, 25, 27, 29}  # chunks built on gpsimd (late, after its memset work)
XBATCH = [4, 6, 6, 8, 8]      # x chunk batching per DMA

_CACHE = {}

# test.py toggles; harness never touches these
TRACE = False
TRACE_DIR = None
LAST_RESULTS = None


def _bank_slot(o):
    return o % 3, o // 3


def build_program():
    import concourse.mybir as mybir
    from concourse import bacc
    from concourse.tile import TileContext

    bf16 = mybir.dt.bfloat16
    f32 = mybir.dt.float32

    nc = bacc.Bacc(trn_type="TRN2")
    # xt pre-interleaved on host: xt[p, k*TPC+t] = x^T[128k+p, t] so every
    # batch DMA is one contiguous per-partition line (big packets)
    xt_d = nc.dram_tensor("xt", (128, NCHUNK * TPC), bf16, kind="ExternalInput")
    vt_d = nc.dram_tensor("vt", (128, NCHUNK * RANK), bf16, kind="ExternalInput")
    s_d = nc.dram_tensor("s_flat", (1, B_IN * SSW), bf16, kind="ExternalInput")
    uc_d = nc.dram_tensor("uc", (96, 3 * BLOCK), bf16, kind="ExternalInput")
    bc_d = nc.dram_tensor("bc", (2, OUT_DIM), bf16, kind="ExternalInput")
    aship_d = nc.dram_tensor("aship", (128, KSHIP * SW), bf16, kind="ExternalInput")
    out_d = nc.dram_tensor("out", (TPC, OUT_DIM), bf16, kind="ExternalOutput")

    with TileContext(nc) as tc:
        from contextlib import ExitStack

        with ExitStack() as ctx:
            consts = ctx.enter_context(tc.tile_pool(name="consts", bufs=1))
            xpool = ctx.enter_context(tc.tile_pool(name="xpool", bufs=1))
            apool = ctx.enter_context(tc.tile_pool(name="apool", bufs=1))
            midsb = ctx.enter_context(tc.tile_pool(name="midsb", bufs=1))
            outsb = ctx.enter_context(tc.tile_pool(name="outsb", bufs=1))
            ps_mid = ctx.enter_context(
                tc.tile_pool(name="ps_mid", bufs=1, space="PSUM")
            )

            # ---- constants / memsets (no DMA deps) ----
            ones_sb = consts.tile([1, TPC], bf16, name="ones_sb", tag="ones_sb")
            nc.vector.memset(ones_sb[:], 1.0)

            # K=128 warm source: HAM only counts full-K matmul activity,
            # K=1 broadcasts leave the clock gate at 1.2 GHz
            wsrc = consts.tile([128, TPC], bf16, name="wsrc", tag="wsrc")
            nc.vector.memset(wsrc[:], 0.0)

            usb = consts.tile([128, OUT_DIM], bf16, name="usb", tag="usb")
            nc.vector.memset(usb[0:112, 0:2048], 0.0)
            nc.gpsimd.memset(usb[0:112, 2048:4096], 0.0)

            # A storage: [128 xrows, chunk k, bank b, col w(128)]
            a_all = apool.tile([128, NCHUNK * SW], bf16, name="a_all", tag="a_all")
            a_v = a_all[:].rearrange("p (k b w) -> p k b w", k=NCHUNK, w=W)
            # built chunks: ones col 112, zero col 113 (rank from muls)
            nc.gpsimd.memset(a_v[:, KSHIP:NCHUNK, :, 112:113], 1.0)
            nc.gpsimd.memset(a_v[:, KSHIP:NCHUNK, :, 113:114], 0.0)

            # ---- input DMAs ----
            # scalar ring: s first (gates stagings), then vt
            s_sb = consts.tile([1, B_IN * SSW], bf16, name="s_sb", tag="s_sb")
            nc.scalar.dma_start(out=s_sb[:], in_=s_d[:])
            vt_sb = consts.tile([128, NCHUNK * RANK], bf16, name="vt_sb", tag="vt_sb")
            nc.scalar.dma_start(out=vt_sb[:], in_=vt_d[:])

            # sync ring (HWDGE): aship FIRST (gates chunk 0), then x batches
            nc.sync.dma_start(out=a_all[:, 0 : KSHIP * SW], in_=aship_d[:])

            xbatches = []
            xoff = []
            k0 = 0
            for nk in XBATCH:
                xb = xpool.tile([128, nk * TPC], bf16, name=f"xb{k0}", tag=f"xb{k0}")
                nc.sync.dma_start(
                    out=xb[:],
                    in_=xt_d[:, k0 * TPC : (k0 + nk) * TPC],
                )
                xbatches.append(xb)
                xoff.append(k0)
                k0 += nk

            # sync ring (idle after the x doorbells): uc + bc for phase B
            for s in range(6):
                n_o = 3 if s < 5 else 1
                nc.sync.dma_start(
                    out=usb[16 * s : 16 * s + 16, 768 * s : 768 * s + 256 * n_o],
                    in_=uc_d[16 * s : 16 * s + 16, 0 : 256 * n_o],
                )
            nc.sync.dma_start(out=usb[112:114, :], in_=bc_d[:])

            def xchunk(k):
                for xb, o in zip(xbatches, xoff):
                    nk = xb.shape[1] // TPC
                    if o <= k < o + nk:
                        return xb[:, (k - o) * TPC : (k - o + 1) * TPC]
                raise AssertionError

            mids = []
            for b in range(NB):
                ms = midsb.tile([128, TPC], bf16, name=f"mids{b}", tag=f"mids{b}")
                mids.append(ms)

            def ap_bsr(t):
                return (
                    t.rearrange("p (b w) -> p b w", b=NB)[:, :, 0:112]
                    .rearrange("p b (s r) -> p b s r", r=RANK)
                )

            with tc.tile_pool(name="ps_pre", bufs=1, space="PSUM") as ps_pre:
                # ---- PE warmup (no DMA deps: ones via memset) ----
                warm = ps_pre.tile([128, TPC], f32, name="warm", tag="warm", bufs=1)
                for _ in range(NWARM):
                    nc.tensor.matmul(
                        warm[:],
                        lhsT=wsrc[:, 0:128],
                        rhs=wsrc[:],
                        start=True,
                        stop=True,
                        tile_position=(0, 0),
                    )

                # ---- open mid banks with 1.0 everywhere ----
                midp = []
                for b in range(NB):
                    mp = ps_mid.tile([128, TPC], f32, name=f"midp{b}", tag=f"midp{b}")
                    nc.tensor.matmul(
                        mp[0:W, :],
                        lhsT=ones_sb[0:1, 0:W],
                        rhs=ones_sb[:],
                        start=True,
                        stop=False,
                        tile_position=(0, 0),
                    )
                    midp.append(mp)

                # ---- phase A: midT accumulation over 32 K-chunks, with the
                # S stagings + A builds interleaved (stage i rides chunk 2i-8)
                stage_at = {}
                for i in range(KSHIP // 2, B_IN):
                    stage_at.setdefault(max(0, 2 * (i - 4)), []).append(i)
                for k in range(NCHUNK):
                    for i in stage_at.get(k, []):
                        sp = ps_pre.tile([128, SSW], f32, name="sp", tag="sp",
                                         bufs=2)
                        nc.tensor.matmul(
                            sp[:],
                            lhsT=ones_sb[0:1, 0:128],
                            rhs=s_sb[0:1, i * SSW : (i + 1) * SSW],
                            start=True,
                            stop=True,
                            tile_position=(0, 0),
                        )
                        # bounce to SBUF at once: frees the PSUM slot fast
                        # (no pool stall on the tensor queue) and lets gpsimd
                        # (no PSUM access) build from it too
                        sps = consts.tile([128, SSW], bf16, name=f"sps{i}",
                                          tag=f"sps{i}")
                        nc.scalar.copy(sps[:], sp[:])
                        for kk in (2 * i, 2 * i + 1):
                            vt_ap = (
                                vt_sb[:, RANK * kk : RANK * (kk + 1)]
                                .unsqueeze(1)
                                .unsqueeze(2)
                                .broadcast_to([128, NB, 7, RANK])
                            )
                            out_ap = a_v[:, kk, :, 0:112].rearrange(
                                "p b (s r) -> p b s r", r=RANK
                            )
                            eng = nc.gpsimd if kk in GPS_BUILD else nc.vector
                            eng.tensor_mul(out_ap, vt_ap, ap_bsr(sps[:]))

                    for b in range(NB):
                        nc.tensor.matmul(
                            midp[b][0:W, :],
                            lhsT=a_v[:, k, b, :],
                            rhs=xchunk(k),
                            start=False,
                            stop=(k == NCHUNK - 1),
                            tile_position=(0, 0),
                        )
                    if k < 20:
                        # cheap full-K keep-warm filler (N=64) through the ramp
                        nc.tensor.matmul(
                            warm[:, 0:64],
                            lhsT=wsrc[:, 0:128],
                            rhs=wsrc[:, 0:64],
                            start=True,
                            stop=True,
                            tile_position=(0, 0),
                        )

                # ---- midT -> SBUF (bf16 cast), token halves for pipelining ----
                for tt in range(2):
                    for b in range(NB):
                        sl = (slice(0, W), slice(tt * 128, (tt + 1) * 128))
                        if (b + tt) % 2 == 0:
                            nc.vector.tensor_copy(mids[b][sl], midp[b][sl])
                        else:
                            nc.scalar.copy(mids[b][sl], midp[b][sl])

            # ---- phase B: per o-block K=114 matmuls (N=256), paired into
            # [128,512] PSUM tiles; flush output pieces as they land ----
            ps_out = ctx.enter_context(
                tc.tile_pool(name="ps_out", bufs=4, space="PSUM")
            )
            for tt in range(2):
                osb_t = outsb.tile(
                    [128, OUT_DIM], bf16, name=f"osb{tt}", tag=f"osb{tt}"
                )
                row = slice(tt * 128, (tt + 1) * 128)
                for j in range(8):
                    po = ps_out.tile([128, 512], f32, name="po", tag="po")
                    for oo in (2 * j, 2 * j + 1):
                        b, _ = _bank_slot(oo)
                        nc.tensor.matmul(
                            po[:, (oo % 2) * BLOCK : (oo % 2 + 1) * BLOCK],
                            lhsT=mids[b][0:W, row],
                            rhs=usb[0:W, BLOCK * oo : BLOCK * (oo + 1)],
                            start=True,
                            stop=True,
                            tile_position=(0, 0),
                        )
                    if j % 2 == 0:
                        nc.vector.tensor_copy(
                            osb_t[:, 512 * j : 512 * (j + 1)], po[:]
                        )
                    else:
                        nc.scalar.copy(
                            osb_t[:, 512 * j : 512 * (j + 1)], po[:]
                        )
                    # 3-piece output flush: 512K + 256K + 256K per token half
                    if j == 3:
                        eng = nc.sync if tt == 0 else nc.gpsimd
                        eng.dma_start(out=out_d[row, 0:2048], in_=osb_t[:, 0:2048])
                    elif j == 5:
                        eng = nc.gpsimd if tt == 0 else nc.sync
                        eng.dma_start(
                            out=out_d[row, 2048:3072], in_=osb_t[:, 2048:3072]
                        )
                    elif j == 7:
                        eng = nc.sync if tt == 0 else nc.gpsimd
                        eng.dma_start(
                            out=out_d[row, 3072:4096], in_=osb_t[:, 3072:4096]
                        )

    nc.compile()
    return nc


def prep_inputs(x, S, U, Vt, bias):
    """Host-side layout prep (bf16). Returns per-core input maps."""
    import ml_dtypes

    bf = ml_dtypes.bfloat16
    x = np.asarray(x, dtype=np.float32)
    S = np.asarray(S, dtype=np.float32)
    U = np.asarray(U, dtype=np.float32)
    Vt = np.asarray(Vt, dtype=np.float32)
    bias = np.asarray(bias, dtype=np.float32)

    # xt[c][p, k*TPC+t] = x_core^T[128k+p, t]: contiguous per-batch lines
    xt = np.ascontiguousarray(x.reshape(TOK, IN_DIM).T).astype(bf)  # (4096, 2048)

    # vt[p, 16k+r] = Vt[i, 128h+p, r], k = 2i+h
    vt_host = np.ascontiguousarray(
        Vt.reshape(B_IN * 2, 128, RANK).transpose(1, 0, 2).reshape(128, NCHUNK * RANK)
    ).astype(bf)

    # s_flat[(i, b, s, r)] = S[o(b,s), i, r]; zero where no o-block
    s_pack = np.zeros((B_IN, NB, W), np.float32)
    for o in range(B_OUT):
        b, s = _bank_slot(o)
        s_pack[:, b, 16 * s : 16 * s + 16] = S[o, :, :]
    s_flat = s_pack.reshape(1, B_IN * SSW).astype(bf)

    # uc[16s+r, 256j+q] = U[3s+j, r, q] (row-group s covers o = 3s..3s+2)
    uc = np.zeros((96, 3 * BLOCK), np.float32)
    for o in range(B_OUT):
        s, j = o // 3, o % 3
        uc[16 * s : 16 * s + 16, BLOCK * j : BLOCK * (j + 1)] = U[o]
    uc = uc.astype(bf)

    # bc row0 = bias (x rowsum row), row1 = comp = -sum_r U[o]
    bc = np.stack([bias, -U.sum(axis=1).reshape(-1)]).astype(bf)

    # shipped A head: chunks 0..KSHIP-1 in [p, k, b, w] layout
    aship = np.zeros((128, KSHIP, NB, W), np.float32)
    for k in range(KSHIP):
        i, h = k // 2, k % 2
        vt_k = Vt[i, 128 * h : 128 * (h + 1), :]  # [128, 16]
        for o in range(B_OUT):
            b, s = _bank_slot(o)
            aship[:, k, b, 16 * s : 16 * s + 16] = vt_k * S[o, i, :][None, :]
        aship[:, k, :, 112] = 1.0
    aship = aship.reshape(128, KSHIP * SW).astype(bf)

    in_maps = []
    for c in range(N_CORES):
        in_maps.append(
            {
                "xt": np.ascontiguousarray(
                    xt[:, c * TPC : (c + 1) * TPC]
                    .reshape(NCHUNK, 128, TPC)
                    .transpose(1, 0, 2)
                    .reshape(128, NCHUNK * TPC)
                ),
                "vt": vt_host,
                "s_flat": s_flat,
                "uc": uc,
                "bc": bc,
                "aship": aship,
            }
        )
    return in_maps


def kernel(x, S, U, Vt, bias):
    global LAST_RESULTS
    from concourse.bass_utils import run_bass_kernel_spmd

    if "nc" not in _CACHE:
        _CACHE["nc"] = build_program()
    nc = _CACHE["nc"]

    in_maps = prep_inputs(x, S, U, Vt, bias)
    res = run_bass_kernel_spmd(
        nc, in_maps, list(range(N_CORES)), trace=TRACE, tmpdir=TRACE_DIR
    )
    LAST_RESULTS = res
    out = np.concatenate(
        [np.asarray(res.results[c]["out"]).astype(np.float32) for c in range(N_CORES)],
        axis=0,
    )
    return out.reshape(2, TOK // 2, OUT_DIM)


# revision 22
# speedup vs baseline: 1.2596x; 1.2596x over previous
"""Trainium2 Bass kernel for nn_Blast: out = x @ (W0 + 1 bias^T) + bias
where W0 block (i_in, i_out) = Vt[i] @ diag(S[o,i]) @ U[o].

v7: full bf16 dataflow (tolerance is 2e-2; bf16 end-to-end lands ~4e-3).

Per core (256 tokens):
  midT[(o,r), tok] = sum_in A[in, (o,r)] * xT[in, tok]     (A = Vt*S, built on device)
  out[tok, oq]     = sum_r midT[(o,r), tok] * U'[o, r, q]

Mid rows pack o-blocks as bank = o%3, slot = o//3: 16 rank rows per slot plus
a shared rowsum row (112; A ones-column -> bias*(rowsum+1) supplies both bias
terms) and a comp row (113; cancels the +1.0 bank-open pollution): bank width
W=114, 3 PSUM banks. Phase A = 32 K-chunks x 3 matmuls (N=256, M=114).

HAM discipline: the PE clock gate (1.2 vs 2.4 GHz) only counts full-K matmul
activity, so the warmup runs K=128 matmuls on a memset tile, phase A is kept
a PURE full-K stream (no K=1 broadcasts: S ships pre-replicated to 128
partitions from the host), N=64 full-K fillers ride along through the DMA
ramp, and extra bridge warms cover the first-x-batch completion latency
(DMA semaphores fire ~2.3us after the last byte lands).

A chunks are built on device: DVE/GPS elementwise muls of vt (shipped
[128p, k, r]) against s_rep slices (shipped [128p, i, (b,slot,r)]), with the
first KSHIP chunks shipped prebuilt. s_rep slices interleave with x batches
on the sync ring so arrival order matches consumption order.

Phase B reads each mid bank wholesale (K=114) against the sparse stacked-U
matrix usb[16s+r, 256o+q] = U[o,r,q] iff s==o//3: one matmul per o-block
(N=256), paired two per PSUM tile so PSUM->SBUF copies run at [128,512].
The mod-3 bank map makes each usb 16-row slot group cover contiguous
o-blocks, so U loads with 6 plain DMAs. Output flushes in 3 pieces per
token half split across sync+gpsimd rings.
"""

import numpy as np

IN_DIM = 4096
OUT_DIM = 4096
BLOCK = 256
RANK = 16
B_IN = 16
B_OUT = 16
N_CORES = 8
TOK = 2048
TPC = TOK // N_CORES          # 256 tokens per core
NCHUNK = IN_DIM // 128        # 32 K-chunks
W = 114                       # mid-bank width: 7*16 rank + rowsum + comp
NB = 3                        # mid banks, o-block -> bank o%3 slot o//3
SW = NB * W                   # 342 A-columns per chunk
KSHIP = 4                     # prebuilt A chunks shipped from host
NWARM = 16                    # K=128 warm matmuls (N=256)
NBRIDGE = 24                  # K=128 N=64 bridge warms over the x-receipt gap
GPS_BUILD = {9, 13, 17, 21, 25, 29}  # chunks built on gpsimd
XBATCH = [4, 6, 6, 8, 8]      # x chunks per DMA
SBATCH = [(2, 8), (8, 12), (12, 16)]  # s_rep [i0, i1) slices after x batches

_CACHE = {}

# test.py toggles; harness never touches these
TRACE = False
TRACE_DIR = None
LAST_RESULTS = None


def _bank_slot(o):
    return o % 3, o // 3


def build_program():
    import concourse.mybir as mybir
    from concourse import bacc
    from concourse.tile import TileContext

    bf16 = mybir.dt.bfloat16
    f32 = mybir.dt.float32

    nc = bacc.Bacc(trn_type="TRN2")
    # xt pre-interleaved on host: xt[p, k*TPC+t] = x^T[128k+p, t] so every
    # batch DMA is one contiguous per-partition line (big packets)
    xt_d = nc.dram_tensor("xt", (128, NCHUNK * TPC), bf16, kind="ExternalInput")
    vt_d = nc.dram_tensor("vt", (128, NCHUNK * RANK), bf16, kind="ExternalInput")
    srep_d = nc.dram_tensor("srep", (128, B_IN * SW), bf16, kind="ExternalInput")
    uc_d = nc.dram_tensor("uc", (96, 3 * BLOCK), bf16, kind="ExternalInput")
    bc_d = nc.dram_tensor("bc", (2, OUT_DIM), bf16, kind="ExternalInput")
    aship_d = nc.dram_tensor("aship", (128, KSHIP * SW), bf16, kind="ExternalInput")
    out_d = nc.dram_tensor("out", (TPC, OUT_DIM), bf16, kind="ExternalOutput")

    with TileContext(nc) as tc:
        from contextlib import ExitStack

        with ExitStack() as ctx:
            consts = ctx.enter_context(tc.tile_pool(name="consts", bufs=1))
            xpool = ctx.enter_context(tc.tile_pool(name="xpool", bufs=1))
            apool = ctx.enter_context(tc.tile_pool(name="apool", bufs=1))
            midsb = ctx.enter_context(tc.tile_pool(name="midsb", bufs=1))
            outsb = ctx.enter_context(tc.tile_pool(name="outsb", bufs=1))
            ps_mid = ctx.enter_context(
                tc.tile_pool(name="ps_mid", bufs=1, space="PSUM")
            )

            # ---- constants / memsets (no DMA deps) ----
            ones_sb = consts.tile([1, TPC], bf16, name="ones_sb", tag="ones_sb")
            nc.vector.memset(ones_sb[:], 1.0)

            # K=128 warm source: HAM only counts full-K matmul activity
            wsrc = consts.tile([128, TPC], bf16, name="wsrc", tag="wsrc")
            nc.vector.memset(wsrc[:], 0.0)

            usb = consts.tile([128, OUT_DIM], bf16, name="usb", tag="usb")
            nc.vector.memset(usb[0:112, 0:2048], 0.0)
            nc.gpsimd.memset(usb[0:112, 2048:4096], 0.0)

            # A storage: [128 xrows, chunk k, bank b, col w]
            a_all = apool.tile([128, NCHUNK * SW], bf16, name="a_all", tag="a_all")
            a_v = a_all[:].rearrange("p (k b w) -> p k b w", k=NCHUNK, w=W)
            # built chunks: ones col 112, zero col 113 (rank cols from muls)
            nc.gpsimd.memset(a_v[:, KSHIP:NCHUNK, :, 112:113], 1.0)
            nc.gpsimd.memset(a_v[:, KSHIP:NCHUNK, :, 113:114], 0.0)

            # ---- input DMAs ----
            # scalar ring: vt + first s_rep slice (gate the early builds)
            vt_sb = consts.tile([128, NCHUNK * RANK], bf16, name="vt_sb", tag="vt_sb")
            nc.scalar.dma_start(out=vt_sb[:], in_=vt_d[:])
            srep = consts.tile([128, B_IN * SW], bf16, name="srep", tag="srep")
            i0, i1 = SBATCH[0]
            nc.scalar.dma_start(
                out=srep[:, i0 * SW : i1 * SW], in_=srep_d[:, i0 * SW : i1 * SW]
            )

            # sync ring: aship, then x batches with s_rep slices interleaved
            nc.sync.dma_start(out=a_all[:, 0 : KSHIP * SW], in_=aship_d[:])
            xbatches = []
            xoff = []
            k0 = 0
            for bi, nk in enumerate(XBATCH):
                xb = xpool.tile([128, nk * TPC], bf16, name=f"xb{k0}", tag=f"xb{k0}")
                nc.sync.dma_start(
                    out=xb[:], in_=xt_d[:, k0 * TPC : (k0 + nk) * TPC]
                )
                xbatches.append(xb)
                xoff.append(k0)
                k0 += nk
                if 1 <= bi <= len(SBATCH) - 1:
                    i0, i1 = SBATCH[bi]
                    nc.sync.dma_start(
                        out=srep[:, i0 * SW : i1 * SW],
                        in_=srep_d[:, i0 * SW : i1 * SW],
                    )

            # sync ring (behind x): uc + bc for phase B
            for s in range(6):
                n_o = 3 if s < 5 else 1
                nc.sync.dma_start(
                    out=usb[16 * s : 16 * s + 16, 768 * s : 768 * s + 256 * n_o],
                    in_=uc_d[16 * s : 16 * s + 16, 0 : 256 * n_o],
                )
            nc.sync.dma_start(out=usb[112:114, :], in_=bc_d[:])

            def xchunk(k):
                for xb, o in zip(xbatches, xoff):
                    nk = xb.shape[1] // TPC
                    if o <= k < o + nk:
                        return xb[:, (k - o) * TPC : (k - o + 1) * TPC]
                raise AssertionError

            mids = []
            for b in range(NB):
                ms = midsb.tile([128, TPC], bf16, name=f"mids{b}", tag=f"mids{b}")
                mids.append(ms)

            # ---- A builds (DVE/GPS): vt x s_rep, no PE involvement ----
            sr_v = srep[:].rearrange("p (i c) -> p i c", i=B_IN)
            for k in range(KSHIP, NCHUNK):
                i = k // 2
                vt_ap = (
                    vt_sb[:, RANK * k : RANK * (k + 1)]
                    .unsqueeze(1)
                    .unsqueeze(2)
                    .broadcast_to([128, NB, 7, RANK])
                )
                s_ap = sr_v[:, i, :].rearrange("p (b w) -> p b w", b=NB)[
                    :, :, 0:112
                ].rearrange("p b (s r) -> p b s r", r=RANK)
                out_ap = a_v[:, k, :, 0:112].rearrange(
                    "p b (s r) -> p b s r", r=RANK
                )
                eng = nc.gpsimd if k in GPS_BUILD else nc.vector
                eng.tensor_mul(out_ap, vt_ap, s_ap)

            with tc.tile_pool(name="ps_pre", bufs=1, space="PSUM") as ps_pre:
                # ---- PE warmup + bridge over the first-x receipt latency ----
                warm = ps_pre.tile([128, TPC], f32, name="warm", tag="warm", bufs=1)
                for _ in range(NWARM):
                    nc.tensor.matmul(
                        warm[:],
                        lhsT=wsrc[:, 0:128],
                        rhs=wsrc[:],
                        start=True,
                        stop=True,
                        tile_position=(0, 0),
                    )
                for _ in range(NBRIDGE):
                    nc.tensor.matmul(
                        warm[:, 0:64],
                        lhsT=wsrc[:, 0:128],
                        rhs=wsrc[:, 0:64],
                        start=True,
                        stop=True,
                        tile_position=(0, 0),
                    )

                # ---- open mid banks with 1.0 everywhere ----
                midp = []
                for b in range(NB):
                    mp = ps_mid.tile([128, TPC], f32, name=f"midp{b}", tag=f"midp{b}")
                    nc.tensor.matmul(
                        mp[0:W, :],
                        lhsT=ones_sb[0:1, 0:W],
                        rhs=ones_sb[:],
                        start=True,
                        stop=False,
                        tile_position=(0, 0),
                    )
                    midp.append(mp)

                # ---- phase A: pure full-K midT accumulation ----
                for k in range(NCHUNK):
                    for b in range(NB):
                        nc.tensor.matmul(
                            midp[b][0:W, :],
                            lhsT=a_v[:, k, b, :],
                            rhs=xchunk(k),
                            start=False,
                            stop=(k == NCHUNK - 1),
                            tile_position=(0, 0),
                        )
                    if k < 20:
                        # cheap full-K keep-warm filler through the ramp
                        nc.tensor.matmul(
                            warm[:, 0:64],
                            lhsT=wsrc[:, 0:128],
                            rhs=wsrc[:, 0:64],
                            start=True,
                            stop=True,
                            tile_position=(0, 0),
                        )

                # ---- midT -> SBUF (bf16 cast), token halves for pipelining ----
                for tt in range(2):
                    for b in range(NB):
                        sl = (slice(0, W), slice(tt * 128, (tt + 1) * 128))
                        if (b + tt) % 2 == 0:
                            nc.vector.tensor_copy(mids[b][sl], midp[b][sl])
                        else:
                            nc.scalar.copy(mids[b][sl], midp[b][sl])

            # ---- phase B: per o-block K=114 matmuls (N=256), paired into
            # [128,512] PSUM tiles; flush output pieces as they land ----
            ps_out = ctx.enter_context(
                tc.tile_pool(name="ps_out", bufs=4, space="PSUM")
            )
            for tt in range(2):
                osb_t = outsb.tile(
                    [128, OUT_DIM], bf16, name=f"osb{tt}", tag=f"osb{tt}"
                )
                row = slice(tt * 128, (tt + 1) * 128)
                for j in range(8):
                    po = ps_out.tile([128, 512], f32, name="po", tag="po")
                    for oo in (2 * j, 2 * j + 1):
                        b, _ = _bank_slot(oo)
                        nc.tensor.matmul(
                            po[:, (oo % 2) * BLOCK : (oo % 2 + 1) * BLOCK],
                            lhsT=mids[b][0:W, row],
                            rhs=usb[0:W, BLOCK * oo : BLOCK * (oo + 1)],
                            start=True,
                            stop=True,
                            tile_position=(0, 0),
                        )
                    if j % 2 == 0:
                        nc.vector.tensor_copy(
                            osb_t[:, 512 * j : 512 * (j + 1)], po[:]
                        )
                    else:
                        nc.scalar.copy(
                            osb_t[:, 512 * j : 512 * (j + 1)], po[:]
                        )
                    # 3-piece output flush: 512K + 256K + 256K per token half
                    if j == 3:
                        eng = nc.sync if tt == 0 else nc.gpsimd
                        eng.dma_start(out=out_d[row, 0:2048], in_=osb_t[:, 0:2048])
                    elif j == 5:
                        eng = nc.gpsimd if tt == 0 else nc.sync
                        eng.dma_start(
                            out=out_d[row, 2048:3072], in_=osb_t[:, 2048:3072]
                        )
                    elif j == 7:
                        eng = nc.sync if tt == 0 else nc.gpsimd
                        eng.dma_start(
                            out=out_d[row, 3072:4096], in_=osb_t[:, 3072:4096]
                        )

    nc.compile()
    return nc


def prep_inputs(x, S, U, Vt, bias):
    """Host-side layout prep (bf16). Returns per-core input maps."""
    import ml_dtypes

    bf = ml_dtypes.bfloat16
    x = np.asarray(x, dtype=np.float32)
    S = np.asarray(S, dtype=np.float32)
    U = np.asarray(U, dtype=np.float32)
    Vt = np.asarray(Vt, dtype=np.float32)
    bias = np.asarray(bias, dtype=np.float32)

    xt = np.ascontiguousarray(x.reshape(TOK, IN_DIM).T).astype(bf)  # (4096, 2048)

    # vt[p, 16k+r] = Vt[i, 128h+p, r], k = 2i+h
    vt_host = np.ascontiguousarray(
        Vt.reshape(B_IN * 2, 128, RANK).transpose(1, 0, 2).reshape(128, NCHUNK * RANK)
    ).astype(bf)

    # s_rep[p, (i, b, s, r)] = S[o(b,s), i, r] replicated over partitions
    s_pack = np.zeros((B_IN, NB, W), np.float32)
    for o in range(B_OUT):
        b, s = _bank_slot(o)
        s_pack[:, b, 16 * s : 16 * s + 16] = S[o, :, :]
    srep = np.ascontiguousarray(
        np.broadcast_to(s_pack.reshape(1, B_IN * SW), (128, B_IN * SW))
    ).astype(bf)

    # uc[16s+r, 256j+q] = U[3s+j, r, q] (row-group s covers o = 3s..3s+2)
    uc = np.zeros((96, 3 * BLOCK), np.float32)
    for o in range(B_OUT):
        s, j = o // 3, o % 3
        uc[16 * s : 16 * s + 16, BLOCK * j : BLOCK * (j + 1)] = U[o]
    uc = uc.astype(bf)

    # bc row0 = bias (x rowsum row), row1 = comp = -sum_r U[o]
    bc = np.stack([bias, -U.sum(axis=1).reshape(-1)]).astype(bf)

    # shipped A head: chunks 0..KSHIP-1 in [p, k, b, w] layout
    aship = np.zeros((128, KSHIP, NB, W), np.float32)
    for k in range(KSHIP):
        i, h = k // 2, k % 2
        vt_k = Vt[i, 128 * h : 128 * (h + 1), :]  # [128, 16]
        for o in range(B_OUT):
            b, s = _bank_slot(o)
            aship[:, k, b, 16 * s : 16 * s + 16] = vt_k * S[o, i, :][None, :]
        aship[:, k, :, 112] = 1.0
    aship = aship.reshape(128, KSHIP * SW).astype(bf)

    in_maps = []
    for c in range(N_CORES):
        in_maps.append(
            {
                "xt": np.ascontiguousarray(
                    xt[:, c * TPC : (c + 1) * TPC]
                    .reshape(NCHUNK, 128, TPC)
                    .transpose(1, 0, 2)
                    .reshape(128, NCHUNK * TPC)
                ),
                "vt": vt_host,
                "srep": srep,
                "uc": uc,
                "bc": bc,
                "aship": aship,
            }
        )
    return in_maps


def kernel(x, S, U, Vt, bias):
    global LAST_RESULTS
    from concourse.bass_utils import run_bass_kernel_spmd

    if "nc" not in _CACHE:
        _CACHE["nc"] = build_program()
    nc = _CACHE["nc"]

    in_maps = prep_inputs(x, S, U, Vt, bias)
    res = run_bass_kernel_spmd(
        nc, in_maps, list(range(N_CORES)), trace=TRACE, tmpdir=TRACE_DIR
    )
    LAST_RESULTS = res
    out = np.concatenate(
        [np.asarray(res.results[c]["out"]).astype(np.float32) for c in range(N_CORES)],
        axis=0,
    )
    return out.reshape(2, TOK // 2, OUT_DIM)


# revision 23
# speedup vs baseline: 1.2684x; 1.0070x over previous
"""Trainium2 Bass kernel for nn_Blast: out = x @ (W0 + 1 bias^T) + bias
where W0 block (i_in, i_out) = Vt[i] @ diag(S[o,i]) @ U[o].

v7: full bf16 dataflow (tolerance is 2e-2; bf16 end-to-end lands ~4e-3).

Per core (256 tokens):
  midT[(o,r), tok] = sum_in A[in, (o,r)] * xT[in, tok]     (A = Vt*S, built on device)
  out[tok, oq]     = sum_r midT[(o,r), tok] * U'[o, r, q]

Mid rows pack o-blocks as bank = o%3, slot = o//3: 16 rank rows per slot plus
a shared rowsum row (112; A ones-column -> bias*(rowsum+1) supplies both bias
terms) and a comp row (113; cancels the +1.0 bank-open pollution): bank width
W=114, 3 PSUM banks. Phase A = 32 K-chunks x 3 matmuls (N=256, M=114).

HAM discipline: the PE clock gate (1.2 vs 2.4 GHz) only counts full-K matmul
activity, so the warmup runs K=128 matmuls on a memset tile, phase A is kept
a PURE full-K stream (no K=1 broadcasts: S ships pre-replicated to 128
partitions from the host), N=64 full-K fillers ride along through the DMA
ramp, and extra bridge warms cover the first-x-batch completion latency
(DMA semaphores fire ~2.3us after the last byte lands).

A chunks are built on device: DVE/GPS elementwise muls of vt (shipped
[128p, k, r]) against s_rep slices (shipped [128p, i, (b,slot,r)]), with the
first KSHIP chunks shipped prebuilt. s_rep slices interleave with x batches
on the sync ring so arrival order matches consumption order.

Phase B reads each mid bank wholesale (K=114) against the sparse stacked-U
matrix usb[16s+r, 256o+q] = U[o,r,q] iff s==o//3: one matmul per o-block
(N=256), paired two per PSUM tile so PSUM->SBUF copies run at [128,512].
The mod-3 bank map makes each usb 16-row slot group cover contiguous
o-blocks, so U loads with 6 plain DMAs. Output flushes in 3 pieces per
token half split across sync+gpsimd rings.
"""

import numpy as np

IN_DIM = 4096
OUT_DIM = 4096
BLOCK = 256
RANK = 16
B_IN = 16
B_OUT = 16
N_CORES = 8
TOK = 2048
TPC = TOK // N_CORES          # 256 tokens per core
NCHUNK = IN_DIM // 128        # 32 K-chunks
W = 114                       # mid-bank width: 7*16 rank + rowsum + comp
NB = 3                        # mid banks, o-block -> bank o%3 slot o//3
SW = NB * W                   # 342 A-columns per chunk
KSHIP = 4                     # prebuilt A chunks shipped from host
NWARM = 24                    # K=128 warm matmuls (N=256): >4us of dense
                              # full-K activity so a HAM window must fire
NBRIDGE = 4                   # extra N=256 warms for arrival jitter
GPS_BUILD = {9, 13, 17, 21, 25, 29}  # chunks built on gpsimd
XBATCH = [6, 6, 6, 7, 7]      # x chunks per DMA
SBATCH = [(2, 8), (8, 12), (12, 16)]  # s_rep [i0, i1) slices after x batches

_CACHE = {}

# test.py toggles; harness never touches these
TRACE = False
TRACE_DIR = None
LAST_RESULTS = None


def _bank_slot(o):
    return o % 3, o // 3


def build_program():
    import concourse.mybir as mybir
    from concourse import bacc
    from concourse.tile import TileContext

    bf16 = mybir.dt.bfloat16
    f32 = mybir.dt.float32

    nc = bacc.Bacc(trn_type="TRN2")
    # xt pre-interleaved on host: xt[p, k*TPC+t] = x^T[128k+p, t] so every
    # batch DMA is one contiguous per-partition line (big packets)
    xt_d = nc.dram_tensor("xt", (128, NCHUNK * TPC), bf16, kind="ExternalInput")
    vt_d = nc.dram_tensor("vt", (128, NCHUNK * RANK), bf16, kind="ExternalInput")
    srep_d = nc.dram_tensor("srep", (128, B_IN * SW), bf16, kind="ExternalInput")
    uc_d = nc.dram_tensor("uc", (96, 3 * BLOCK), bf16, kind="ExternalInput")
    bc_d = nc.dram_tensor("bc", (2, OUT_DIM), bf16, kind="ExternalInput")
    aship_d = nc.dram_tensor("aship", (128, KSHIP * SW), bf16, kind="ExternalInput")
    out_d = nc.dram_tensor("out", (TPC, OUT_DIM), bf16, kind="ExternalOutput")

    with TileContext(nc) as tc:
        from contextlib import ExitStack

        with ExitStack() as ctx:
            consts = ctx.enter_context(tc.tile_pool(name="consts", bufs=1))
            xpool = ctx.enter_context(tc.tile_pool(name="xpool", bufs=1))
            apool = ctx.enter_context(tc.tile_pool(name="apool", bufs=1))
            midsb = ctx.enter_context(tc.tile_pool(name="midsb", bufs=1))
            outsb = ctx.enter_context(tc.tile_pool(name="outsb", bufs=1))
            ps_mid = ctx.enter_context(
                tc.tile_pool(name="ps_mid", bufs=1, space="PSUM")
            )

            # ---- constants / memsets (no DMA deps) ----
            ones_sb = consts.tile([1, TPC], bf16, name="ones_sb", tag="ones_sb")
            nc.vector.memset(ones_sb[:], 1.0)

            # K=128 warm source: HAM only counts full-K matmul activity
            wsrc = consts.tile([128, TPC], bf16, name="wsrc", tag="wsrc")
            nc.vector.memset(wsrc[:], 0.0)

            usb = consts.tile([128, OUT_DIM], bf16, name="usb", tag="usb")
            nc.vector.memset(usb[0:112, 0:2048], 0.0)
            nc.gpsimd.memset(usb[0:112, 2048:4096], 0.0)

            # A storage: [128 xrows, chunk k, bank b, col w]
            a_all = apool.tile([128, NCHUNK * SW], bf16, name="a_all", tag="a_all")
            a_v = a_all[:].rearrange("p (k b w) -> p k b w", k=NCHUNK, w=W)
            # built chunks: ones col 112, zero col 113 (rank cols from muls)
            nc.gpsimd.memset(a_v[:, KSHIP:NCHUNK, :, 112:113], 1.0)
            nc.gpsimd.memset(a_v[:, KSHIP:NCHUNK, :, 113:114], 0.0)

            # ---- input DMAs ----
            # scalar ring: vt + first s_rep slice (gate the early builds)
            vt_sb = consts.tile([128, NCHUNK * RANK], bf16, name="vt_sb", tag="vt_sb")
            nc.scalar.dma_start(out=vt_sb[:], in_=vt_d[:])
            srep = consts.tile([128, B_IN * SW], bf16, name="srep", tag="srep")
            i0, i1 = SBATCH[0]
            nc.scalar.dma_start(
                out=srep[:, i0 * SW : i1 * SW], in_=srep_d[:, i0 * SW : i1 * SW]
            )

            # sync ring: aship, then x batches with s_rep slices interleaved
            nc.sync.dma_start(out=a_all[:, 0 : KSHIP * SW], in_=aship_d[:])
            xbatches = []
            xoff = []
            k0 = 0
            for bi, nk in enumerate(XBATCH):
                xb = xpool.tile([128, nk * TPC], bf16, name=f"xb{k0}", tag=f"xb{k0}")
                nc.sync.dma_start(
                    out=xb[:], in_=xt_d[:, k0 * TPC : (k0 + nk) * TPC]
                )
                xbatches.append(xb)
                xoff.append(k0)
                k0 += nk
                if 1 <= bi <= len(SBATCH) - 1:
                    i0, i1 = SBATCH[bi]
                    nc.sync.dma_start(
                        out=srep[:, i0 * SW : i1 * SW],
                        in_=srep_d[:, i0 * SW : i1 * SW],
                    )

            # sync ring (behind x): uc + bc for phase B
            for s in range(6):
                n_o = 3 if s < 5 else 1
                nc.sync.dma_start(
                    out=usb[16 * s : 16 * s + 16, 768 * s : 768 * s + 256 * n_o],
                    in_=uc_d[16 * s : 16 * s + 16, 0 : 256 * n_o],
                )
            nc.sync.dma_start(out=usb[112:114, :], in_=bc_d[:])

            def xchunk(k):
                for xb, o in zip(xbatches, xoff):
                    nk = xb.shape[1] // TPC
                    if o <= k < o + nk:
                        return xb[:, (k - o) * TPC : (k - o + 1) * TPC]
                raise AssertionError

            mids = []
            for b in range(NB):
                ms = midsb.tile([128, TPC], bf16, name=f"mids{b}", tag=f"mids{b}")
                mids.append(ms)

            # ---- A builds (DVE/GPS): vt x s_rep, no PE involvement ----
            sr_v = srep[:].rearrange("p (i c) -> p i c", i=B_IN)
            for k in range(KSHIP, NCHUNK):
                i = k // 2
                vt_ap = (
                    vt_sb[:, RANK * k : RANK * (k + 1)]
                    .unsqueeze(1)
                    .unsqueeze(2)
                    .broadcast_to([128, NB, 7, RANK])
                )
                s_ap = sr_v[:, i, :].rearrange("p (b w) -> p b w", b=NB)[
                    :, :, 0:112
                ].rearrange("p b (s r) -> p b s r", r=RANK)
                out_ap = a_v[:, k, :, 0:112].rearrange(
                    "p b (s r) -> p b s r", r=RANK
                )
                eng = nc.gpsimd if k in GPS_BUILD else nc.vector
                eng.tensor_mul(out_ap, vt_ap, s_ap)

            with tc.tile_pool(name="ps_pre", bufs=1, space="PSUM") as ps_pre:
                # ---- PE warmup + bridge over the first-x receipt latency ----
                warm = ps_pre.tile([128, TPC], f32, name="warm", tag="warm", bufs=1)
                for _ in range(NWARM):
                    nc.tensor.matmul(
                        warm[:],
                        lhsT=wsrc[:, 0:128],
                        rhs=wsrc[:],
                        start=True,
                        stop=True,
                        tile_position=(0, 0),
                    )
                for _ in range(NBRIDGE):
                    nc.tensor.matmul(
                        warm[:],
                        lhsT=wsrc[:, 0:128],
                        rhs=wsrc[:],
                        start=True,
                        stop=True,
                        tile_position=(0, 0),
                    )

                # ---- open mid banks with 1.0 everywhere ----
                midp = []
                for b in range(NB):
                    mp = ps_mid.tile([128, TPC], f32, name=f"midp{b}", tag=f"midp{b}")
                    nc.tensor.matmul(
                        mp[0:W, :],
                        lhsT=ones_sb[0:1, 0:W],
                        rhs=ones_sb[:],
                        start=True,
                        stop=False,
                        tile_position=(0, 0),
                    )
                    midp.append(mp)

                # ---- phase A: pure full-K midT accumulation ----
                for k in range(NCHUNK):
                    for b in range(NB):
                        nc.tensor.matmul(
                            midp[b][0:W, :],
                            lhsT=a_v[:, k, b, :],
                            rhs=xchunk(k),
                            start=False,
                            stop=(k == NCHUNK - 1),
                            tile_position=(0, 0),
                        )
                    if k < 20:
                        # cheap full-K keep-warm filler through the ramp
                        nc.tensor.matmul(
                            warm[:, 0:64],
                            lhsT=wsrc[:, 0:128],
                            rhs=wsrc[:, 0:64],
                            start=True,
                            stop=True,
                            tile_position=(0, 0),
                        )

                # ---- midT -> SBUF (bf16 cast), token halves for pipelining ----
                for tt in range(2):
                    for b in range(NB):
                        sl = (slice(0, W), slice(tt * 128, (tt + 1) * 128))
                        if (b + tt) % 2 == 0:
                            nc.vector.tensor_copy(mids[b][sl], midp[b][sl])
                        else:
                            nc.scalar.copy(mids[b][sl], midp[b][sl])

            # ---- phase B: per o-block K=114 matmuls (N=256), paired into
            # [128,512] PSUM tiles; flush output pieces as they land ----
            ps_out = ctx.enter_context(
                tc.tile_pool(name="ps_out", bufs=4, space="PSUM")
            )
            for tt in range(2):
                osb_t = outsb.tile(
                    [128, OUT_DIM], bf16, name=f"osb{tt}", tag=f"osb{tt}"
                )
                row = slice(tt * 128, (tt + 1) * 128)
                for j in range(8):
                    po = ps_out.tile([128, 512], f32, name="po", tag="po")
                    for oo in (2 * j, 2 * j + 1):
                        b, _ = _bank_slot(oo)
                        nc.tensor.matmul(
                            po[:, (oo % 2) * BLOCK : (oo % 2 + 1) * BLOCK],
                            lhsT=mids[b][0:W, row],
                            rhs=usb[0:W, BLOCK * oo : BLOCK * (oo + 1)],
                            start=True,
                            stop=True,
                            tile_position=(0, 0),
                        )
                    if j % 2 == 0:
                        nc.vector.tensor_copy(
                            osb_t[:, 512 * j : 512 * (j + 1)], po[:]
                        )
                    else:
                        nc.scalar.copy(
                            osb_t[:, 512 * j : 512 * (j + 1)], po[:]
                        )
                    # 4-piece output flush (256KB each), alternating rings
                    if j in (1, 3, 5, 7):
                        eng = nc.sync if (j // 2 + tt) % 2 == 0 else nc.gpsimd
                        c0 = 512 * (j - 1)
                        eng.dma_start(
                            out=out_d[row, c0 : c0 + 1024],
                            in_=osb_t[:, c0 : c0 + 1024],
                        )

    nc.compile()
    return nc


def prep_inputs(x, S, U, Vt, bias):
    """Host-side layout prep (bf16). Returns per-core input maps."""
    import ml_dtypes

    bf = ml_dtypes.bfloat16
    x = np.asarray(x, dtype=np.float32)
    S = np.asarray(S, dtype=np.float32)
    U = np.asarray(U, dtype=np.float32)
    Vt = np.asarray(Vt, dtype=np.float32)
    bias = np.asarray(bias, dtype=np.float32)

    xt = np.ascontiguousarray(x.reshape(TOK, IN_DIM).T).astype(bf)  # (4096, 2048)

    # vt[p, 16k+r] = Vt[i, 128h+p, r], k = 2i+h
    vt_host = np.ascontiguousarray(
        Vt.reshape(B_IN * 2, 128, RANK).transpose(1, 0, 2).reshape(128, NCHUNK * RANK)
    ).astype(bf)

    # s_rep[p, (i, b, s, r)] = S[o(b,s), i, r] replicated over partitions
    s_pack = np.zeros((B_IN, NB, W), np.float32)
    for o in range(B_OUT):
        b, s = _bank_slot(o)
        s_pack[:, b, 16 * s : 16 * s + 16] = S[o, :, :]
    srep = np.ascontiguousarray(
        np.broadcast_to(s_pack.reshape(1, B_IN * SW), (128, B_IN * SW))
    ).astype(bf)

    # uc[16s+r, 256j+q] = U[3s+j, r, q] (row-group s covers o = 3s..3s+2)
    uc = np.zeros((96, 3 * BLOCK), np.float32)
    for o in range(B_OUT):
        s, j = o // 3, o % 3
        uc[16 * s : 16 * s + 16, BLOCK * j : BLOCK * (j + 1)] = U[o]
    uc = uc.astype(bf)

    # bc row0 = bias (x rowsum row), row1 = comp = -sum_r U[o]
    bc = np.stack([bias, -U.sum(axis=1).reshape(-1)]).astype(bf)

    # shipped A head: chunks 0..KSHIP-1 in [p, k, b, w] layout
    aship = np.zeros((128, KSHIP, NB, W), np.float32)
    for k in range(KSHIP):
        i, h = k // 2, k % 2
        vt_k = Vt[i, 128 * h : 128 * (h + 1), :]  # [128, 16]
        for o in range(B_OUT):
            b, s = _bank_slot(o)
            aship[:, k, b, 16 * s : 16 * s + 16] = vt_k * S[o, i, :][None, :]
        aship[:, k, :, 112] = 1.0
    aship = aship.reshape(128, KSHIP * SW).astype(bf)

    in_maps = []
    for c in range(N_CORES):
        in_maps.append(
            {
                "xt": np.ascontiguousarray(
                    xt[:, c * TPC : (c + 1) * TPC]
                    .reshape(NCHUNK, 128, TPC)
                    .transpose(1, 0, 2)
                    .reshape(128, NCHUNK * TPC)
                ),
                "vt": vt_host,
                "srep": srep,
                "uc": uc,
                "bc": bc,
                "aship": aship,
            }
        )
    return in_maps


def kernel(x, S, U, Vt, bias):
    global LAST_RESULTS
    from concourse.bass_utils import run_bass_kernel_spmd

    if "nc" not in _CACHE:
        _CACHE["nc"] = build_program()
    nc = _CACHE["nc"]

    in_maps = prep_inputs(x, S, U, Vt, bias)
    res = run_bass_kernel_spmd(
        nc, in_maps, list(range(N_CORES)), trace=TRACE, tmpdir=TRACE_DIR
    )
    LAST_RESULTS = res
    out = np.concatenate(
        [np.asarray(res.results[c]["out"]).astype(np.float32) for c in range(N_CORES)],
        axis=0,
    )
    return out.reshape(2, TOK // 2, OUT_DIM)


# revision 25
# speedup vs baseline: 1.4119x; 1.1131x over previous
"""Trainium2 Bass kernel for nn_Blast: out = x @ (W0 + 1 bias^T) + bias
where W0 block (i_in, i_out) = Vt[i] @ diag(S[o,i]) @ U[o].

v7: full bf16 dataflow (tolerance is 2e-2; bf16 end-to-end lands ~4e-3).

Per core (256 tokens):
  midT[(o,r), tok] = sum_in A[in, (o,r)] * xT[in, tok]     (A = Vt*S, built on device)
  out[tok, oq]     = sum_r midT[(o,r), tok] * U'[o, r, q]

Mid rows pack o-blocks as bank = o%3, slot = o//3: 16 rank rows per slot plus
a shared rowsum row (112; A ones-column -> bias*(rowsum+1) supplies both bias
terms) and a comp row (113; cancels the +1.0 bank-open pollution): bank width
W=114, 3 PSUM banks. Phase A = 32 K-chunks x 3 matmuls (N=256, M=114).

HAM discipline: the PE clock gate (1.2 vs 2.4 GHz) only counts full-K matmul
activity, so the warmup runs K=128 matmuls on a memset tile, phase A is kept
a PURE full-K stream (no K=1 broadcasts: S ships pre-replicated to 128
partitions from the host), N=64 full-K fillers ride along through the DMA
ramp, and extra bridge warms cover the first-x-batch completion latency
(DMA semaphores fire ~2.3us after the last byte lands).

A chunks are built on device: DVE/GPS elementwise muls of vt (shipped
[128p, k, r]) against s_rep slices (shipped [128p, i, (b,slot,r)]), with the
first KSHIP chunks shipped prebuilt. s_rep slices interleave with x batches
on the sync ring so arrival order matches consumption order.

Phase B reads each mid bank wholesale (K=114) against the sparse stacked-U
matrix usb[16s+r, 256o+q] = U[o,r,q] iff s==o//3: one matmul per o-block
(N=256), paired two per PSUM tile so PSUM->SBUF copies run at [128,512].
The mod-3 bank map makes each usb 16-row slot group cover contiguous
o-blocks, so U loads with 6 plain DMAs. Output flushes in 3 pieces per
token half split across sync+gpsimd rings.
"""

import numpy as np

IN_DIM = 4096
OUT_DIM = 4096
BLOCK = 256
RANK = 16
B_IN = 16
B_OUT = 16
N_CORES = 8
TOK = 2048
TPC = TOK // N_CORES          # 256 tokens per core
NCHUNK = IN_DIM // 128        # 32 K-chunks
W = 114                       # mid-bank width: 7*16 rank + rowsum + comp
NB = 3                        # mid banks, o-block -> bank o%3 slot o//3
SW = NB * W                   # 342 A-columns per chunk
KSHIP = 4                     # prebuilt A chunks shipped from host
NWARM = 20                    # K=128 warm matmuls (N=256): >4us of dense
                              # full-K activity so a HAM window must fire
NBRIDGE = 4                   # extra N=256 warms for arrival jitter
GPS_BUILD = {9, 13, 17, 21, 25, 29}  # chunks built on gpsimd
XBATCH = [1, 5, 6, 8, 8, 4]   # x chunks per DMA (tiny canary first: its
                              # completion receipt gates phase-A start)
SBATCH = [(2, 4), (4, 8), (8, 12), (12, 16)]  # s_rep [i0, i1) slices

_CACHE = {}

# test.py toggles; harness never touches these
TRACE = False
TRACE_DIR = None
LAST_RESULTS = None


def _bank_slot(o):
    return o % 3, o // 3


def build_program():
    import concourse.mybir as mybir
    from concourse import bacc
    from concourse.tile import TileContext

    bf16 = mybir.dt.bfloat16
    f32 = mybir.dt.float32

    nc = bacc.Bacc(trn_type="TRN2")
    # xt pre-interleaved on host: xt[p, k*TPC+t] = x^T[128k+p, t] so every
    # batch DMA is one contiguous per-partition line (big packets)
    xt_d = nc.dram_tensor("xt", (128, NCHUNK * TPC), bf16, kind="ExternalInput")
    vt_d = nc.dram_tensor("vt", (128, NCHUNK * RANK), bf16, kind="ExternalInput")
    srep_d = nc.dram_tensor("srep", (128, B_IN * SW), bf16, kind="ExternalInput")
    uc_d = nc.dram_tensor("uc", (96, 3 * BLOCK), bf16, kind="ExternalInput")
    bc_d = nc.dram_tensor("bc", (2, OUT_DIM), bf16, kind="ExternalInput")
    aship_d = nc.dram_tensor("aship", (128, KSHIP * SW), bf16, kind="ExternalInput")
    out_d = nc.dram_tensor("out", (TPC, OUT_DIM), bf16, kind="ExternalOutput")

    with TileContext(nc) as tc:
        from contextlib import ExitStack

        with ExitStack() as ctx:
            consts = ctx.enter_context(tc.tile_pool(name="consts", bufs=1))
            xpool = ctx.enter_context(tc.tile_pool(name="xpool", bufs=1))
            apool = ctx.enter_context(tc.tile_pool(name="apool", bufs=1))
            midsb = ctx.enter_context(tc.tile_pool(name="midsb", bufs=1))
            outsb = ctx.enter_context(tc.tile_pool(name="outsb", bufs=1))
            ps_mid = ctx.enter_context(
                tc.tile_pool(name="ps_mid", bufs=1, space="PSUM")
            )

            # ---- constants / memsets (no DMA deps) ----
            ones_sb = consts.tile([1, TPC], bf16, name="ones_sb", tag="ones_sb")
            nc.vector.memset(ones_sb[:], 1.0)

            # K=128 warm source: HAM only counts full-K matmul activity
            wsrc = consts.tile([128, TPC], bf16, name="wsrc", tag="wsrc")
            nc.vector.memset(wsrc[:], 0.0)

            usb = consts.tile([128, OUT_DIM], bf16, name="usb", tag="usb")
            nc.vector.memset(usb[0:112, 0:2048], 0.0)
            nc.gpsimd.memset(usb[0:112, 2048:4096], 0.0)

            # A storage: [128 xrows, chunk k, bank b, col w]
            a_all = apool.tile([128, NCHUNK * SW], bf16, name="a_all", tag="a_all")
            a_v = a_all[:].rearrange("p (k b w) -> p k b w", k=NCHUNK, w=W)
            # built chunks: ones col 112, zero col 113 (rank cols from muls)
            nc.gpsimd.memset(a_v[:, KSHIP:NCHUNK, :, 112:113], 1.0)
            nc.gpsimd.memset(a_v[:, KSHIP:NCHUNK, :, 113:114], 0.0)

            # ---- input DMAs ----
            # scalar ring: vt + first s_rep slice (gate the early builds)
            vt_sb = consts.tile([128, NCHUNK * RANK], bf16, name="vt_sb", tag="vt_sb")
            nc.scalar.dma_start(out=vt_sb[:], in_=vt_d[:])
            srep = consts.tile([128, B_IN * SW], bf16, name="srep", tag="srep")
            i0, i1 = SBATCH[0]
            nc.scalar.dma_start(
                out=srep[:, i0 * SW : i1 * SW], in_=srep_d[:, i0 * SW : i1 * SW]
            )

            # sync ring: aship, then x batches with s_rep slices interleaved
            nc.sync.dma_start(out=a_all[:, 0 : KSHIP * SW], in_=aship_d[:])
            xbatches = []
            xoff = []
            k0 = 0
            for bi, nk in enumerate(XBATCH):
                xb = xpool.tile([128, nk * TPC], bf16, name=f"xb{k0}", tag=f"xb{k0}")
                nc.sync.dma_start(
                    out=xb[:], in_=xt_d[:, k0 * TPC : (k0 + nk) * TPC]
                )
                xbatches.append(xb)
                xoff.append(k0)
                k0 += nk
                if 2 <= bi <= len(SBATCH):
                    i0, i1 = SBATCH[bi - 1]
                    nc.sync.dma_start(
                        out=srep[:, i0 * SW : i1 * SW],
                        in_=srep_d[:, i0 * SW : i1 * SW],
                    )

            # sync ring (behind x): uc + bc for phase B
            for s in range(6):
                n_o = 3 if s < 5 else 1
                nc.sync.dma_start(
                    out=usb[16 * s : 16 * s + 16, 768 * s : 768 * s + 256 * n_o],
                    in_=uc_d[16 * s : 16 * s + 16, 0 : 256 * n_o],
                )
            nc.sync.dma_start(out=usb[112:114, :], in_=bc_d[:])

            def xchunk(k):
                for xb, o in zip(xbatches, xoff):
                    nk = xb.shape[1] // TPC
                    if o <= k < o + nk:
                        return xb[:, (k - o) * TPC : (k - o + 1) * TPC]
                raise AssertionError

            mids = []
            for b in range(NB):
                ms = midsb.tile([128, TPC], bf16, name=f"mids{b}", tag=f"mids{b}")
                mids.append(ms)

            # ---- A builds (DVE/GPS): vt x s_rep, no PE involvement ----
            sr_v = srep[:].rearrange("p (i c) -> p i c", i=B_IN)
            for k in range(KSHIP, NCHUNK):
                i = k // 2
                vt_ap = (
                    vt_sb[:, RANK * k : RANK * (k + 1)]
                    .unsqueeze(1)
                    .unsqueeze(2)
                    .broadcast_to([128, NB, 7, RANK])
                )
                s_ap = sr_v[:, i, :].rearrange("p (b w) -> p b w", b=NB)[
                    :, :, 0:112
                ].rearrange("p b (s r) -> p b s r", r=RANK)
                out_ap = a_v[:, k, :, 0:112].rearrange(
                    "p b (s r) -> p b s r", r=RANK
                )
                eng = nc.gpsimd if k in GPS_BUILD else nc.vector
                eng.tensor_mul(out_ap, vt_ap, s_ap)

            with tc.tile_pool(name="ps_pre", bufs=1, space="PSUM") as ps_pre:
                # ---- PE warmup + bridge over the first-x receipt latency ----
                warm = ps_pre.tile([128, TPC], f32, name="warm", tag="warm", bufs=1)
                for _ in range(NWARM):
                    nc.tensor.matmul(
                        warm[:],
                        lhsT=wsrc[:, 0:128],
                        rhs=wsrc[:],
                        start=True,
                        stop=True,
                        tile_position=(0, 0),
                    )
                for _ in range(NBRIDGE):
                    nc.tensor.matmul(
                        warm[:],
                        lhsT=wsrc[:, 0:128],
                        rhs=wsrc[:],
                        start=True,
                        stop=True,
                        tile_position=(0, 0),
                    )

                # ---- open mid banks with 1.0 everywhere ----
                midp = []
                for b in range(NB):
                    mp = ps_mid.tile([128, TPC], f32, name=f"midp{b}", tag=f"midp{b}")
                    nc.tensor.matmul(
                        mp[0:W, :],
                        lhsT=ones_sb[0:1, 0:W],
                        rhs=ones_sb[:],
                        start=True,
                        stop=False,
                        tile_position=(0, 0),
                    )
                    midp.append(mp)

                # ---- phase A: pure full-K midT accumulation ----
                for k in range(NCHUNK):
                    for b in range(NB):
                        nc.tensor.matmul(
                            midp[b][0:W, :],
                            lhsT=a_v[:, k, b, :],
                            rhs=xchunk(k),
                            start=False,
                            stop=(k == NCHUNK - 1),
                            tile_position=(0, 0),
                        )
                    if k < 20:
                        # cheap full-K keep-warm filler through the ramp
                        nc.tensor.matmul(
                            warm[:, 0:64],
                            lhsT=wsrc[:, 0:128],
                            rhs=wsrc[:, 0:64],
                            start=True,
                            stop=True,
                            tile_position=(0, 0),
                        )

                # ---- midT -> SBUF (bf16 cast), token halves for pipelining ----
                for tt in range(2):
                    for b in range(NB):
                        sl = (slice(0, W), slice(tt * 128, (tt + 1) * 128))
                        if (b + tt) % 2 == 0:
                            nc.vector.tensor_copy(mids[b][sl], midp[b][sl])
                        else:
                            nc.scalar.copy(mids[b][sl], midp[b][sl])

            # ---- phase B: per o-block K=114 matmuls (N=256), paired into
            # [128,512] PSUM tiles; flush output pieces as they land ----
            ps_out = ctx.enter_context(
                tc.tile_pool(name="ps_out", bufs=4, space="PSUM")
            )
            for tt in range(2):
                osb_t = outsb.tile(
                    [128, OUT_DIM], bf16, name=f"osb{tt}", tag=f"osb{tt}"
                )
                row = slice(tt * 128, (tt + 1) * 128)
                for j in range(8):
                    po = ps_out.tile([128, 512], f32, name="po", tag="po")
                    for oo in (2 * j, 2 * j + 1):
                        b, _ = _bank_slot(oo)
                        nc.tensor.matmul(
                            po[:, (oo % 2) * BLOCK : (oo % 2 + 1) * BLOCK],
                            lhsT=mids[b][0:W, row],
                            rhs=usb[0:W, BLOCK * oo : BLOCK * (oo + 1)],
                            start=True,
                            stop=True,
                            tile_position=(0, 0),
                        )
                    dst = osb_t[:, 512 * j : 512 * (j + 1)]
                    if j % 2 == 0:
                        nc.vector.tensor_copy(dst, po[:])
                    else:
                        nc.scalar.copy(dst, po[:])
                    # 4-piece output flush (256KB each) on the idle sync ring
                    if j in (1, 3, 5, 7):
                        c0 = 512 * (j - 1)
                        nc.sync.dma_start(
                            out=out_d[row, c0 : c0 + 1024],
                            in_=osb_t[:, c0 : c0 + 1024],
                        )

    nc.compile()
    return nc


def prep_inputs(x, S, U, Vt, bias):
    """Host-side layout prep (bf16). Returns per-core input maps."""
    import ml_dtypes

    bf = ml_dtypes.bfloat16
    x = np.asarray(x, dtype=np.float32)
    S = np.asarray(S, dtype=np.float32)
    U = np.asarray(U, dtype=np.float32)
    Vt = np.asarray(Vt, dtype=np.float32)
    bias = np.asarray(bias, dtype=np.float32)

    xt = np.ascontiguousarray(x.reshape(TOK, IN_DIM).T).astype(bf)  # (4096, 2048)

    # vt[p, 16k+r] = Vt[i, 128h+p, r], k = 2i+h
    vt_host = np.ascontiguousarray(
        Vt.reshape(B_IN * 2, 128, RANK).transpose(1, 0, 2).reshape(128, NCHUNK * RANK)
    ).astype(bf)

    # s_rep[p, (i, b, s, r)] = S[o(b,s), i, r] replicated over partitions
    s_pack = np.zeros((B_IN, NB, W), np.float32)
    for o in range(B_OUT):
        b, s = _bank_slot(o)
        s_pack[:, b, 16 * s : 16 * s + 16] = S[o, :, :]
    srep = np.ascontiguousarray(
        np.broadcast_to(s_pack.reshape(1, B_IN * SW), (128, B_IN * SW))
    ).astype(bf)

    # uc[16s+r, 256j+q] = U[3s+j, r, q] (row-group s covers o = 3s..3s+2)
    uc = np.zeros((96, 3 * BLOCK), np.float32)
    for o in range(B_OUT):
        s, j = o // 3, o % 3
        uc[16 * s : 16 * s + 16, BLOCK * j : BLOCK * (j + 1)] = U[o]
    uc = uc.astype(bf)

    # bc row0 = bias (x rowsum row), row1 = comp = -sum_r U[o]
    bc = np.stack([bias, -U.sum(axis=1).reshape(-1)]).astype(bf)

    # shipped A head: chunks 0..KSHIP-1 in [p, k, b, w] layout
    aship = np.zeros((128, KSHIP, NB, W), np.float32)
    for k in range(KSHIP):
        i, h = k // 2, k % 2
        vt_k = Vt[i, 128 * h : 128 * (h + 1), :]  # [128, 16]
        for o in range(B_OUT):
            b, s = _bank_slot(o)
            aship[:, k, b, 16 * s : 16 * s + 16] = vt_k * S[o, i, :][None, :]
        aship[:, k, :, 112] = 1.0
    aship = aship.reshape(128, KSHIP * SW).astype(bf)

    in_maps = []
    for c in range(N_CORES):
        in_maps.append(
            {
                "xt": np.ascontiguousarray(
                    xt[:, c * TPC : (c + 1) * TPC]
                    .reshape(NCHUNK, 128, TPC)
                    .transpose(1, 0, 2)
                    .reshape(128, NCHUNK * TPC)
                ),
                "vt": vt_host,
                "srep": srep,
                "uc": uc,
                "bc": bc,
                "aship": aship,
            }
        )
    return in_maps


def kernel(x, S, U, Vt, bias):
    global LAST_RESULTS
    from concourse.bass_utils import run_bass_kernel_spmd

    if "nc" not in _CACHE:
        _CACHE["nc"] = build_program()
    nc = _CACHE["nc"]

    in_maps = prep_inputs(x, S, U, Vt, bias)
    res = run_bass_kernel_spmd(
        nc, in_maps, list(range(N_CORES)), trace=TRACE, tmpdir=TRACE_DIR
    )
    LAST_RESULTS = res
    out = np.concatenate(
        [np.asarray(res.results[c]["out"]).astype(np.float32) for c in range(N_CORES)],
        axis=0,
    )
    return out.reshape(2, TOK // 2, OUT_DIM)
